# revision 9
# baseline (speedup 1.0000x reference)
"""Trainium2 Bass kernel for nn_AttentionBlock (GroupNorm + single-head
self-attention + proj + residual), data-parallel over batch on 8 cores.

Contract: kernel(**inputs) takes the FULL unsharded inputs
  x (8, 256, 64, 64) f32, gn_scale (256,), gn_bias (256,),
  qkv_w (768, 256), qkv_b (768,), proj_w (256, 256), proj_b (256,)
and returns the FULL output (8, 256, 64, 64) f32.

Per-core plan (one sample per core):
  - x viewed as (C=256, N=4096) = (channels on partitions, tokens on free dim)
  - GroupNorm(8 groups) stats via bn_stats/bn_aggr + tiny indicator matmuls
  - xn cast to bf16; QKV as channel matmuls:
      Q, K produced in (d, n) layout  [d on partitions]
      V produced in token-major (n, d) layout
    so that attention needs NO transposes:
      scoresT[k, q] = sum_d K[d,k] Q[d,q]   (lhsT=K tile, rhs=Q tile)
      PT = exp(scoresT)                     (softmax scale folded into Q)
      out_un[d, q] = sum_k V[k,d] PT[k,q]   (lhsT=V tile, rhs=PT)
      denom[q] = sum_k PT[k,q]  -> DVE accumulation + one ones-matmul that
                                   also broadcasts the sum over partitions
  - out = proj(out_un * 1/denom) + proj_b + x  (residual in f32)
"""

import os
import sys

import numpy as np

for _p in (
    "/opt/trn_rl_repo",
    "/root/.axon_site",
    "/root/.axon_site/_ro/trn_rl_repo",
    "/root/.axon_site/_ro/pypackages",
):
    if os.path.isdir(_p) and _p not in sys.path:
        sys.path.append(_p)

import ml_dtypes  # noqa: E402

import concourse.bass as bass  # noqa: E402
import concourse.mybir as mybir  # noqa: E402
import concourse.tile as tile  # noqa: E402
from concourse import bacc  # noqa: E402

F32 = mybir.dt.float32
BF16 = mybir.dt.bfloat16
AF = mybir.ActivationFunctionType
ALU = mybir.AluOpType

B, C, H, W = 8, 256, 64, 64
GROUPS = 8
EPS = 1e-5
P = 128
N_CORES = 8
ATT_SCALE = float(C) ** -0.5  # 1/16


def build_nc(n_tok=H * W):
    """Build the single-core Bass program (SPMD across 8 cores)."""
    CCH = C // P            # channel chunks (2)
    QT = 512                # q-tile width (one PSUM bank of f32)
    NQ = n_tok // QT        # number of q tiles
    NKB = n_tok // P        # number of 128-token key blocks
    GSZ = C // GROUPS       # channels per group (32)
    G_PER_CHUNK = GROUPS // CCH  # groups per 128-channel chunk (4)

    # Bacc (not plain Bass): its compile() runs move_matmul_waits_to_ldweights
    # + generate_event_semaphores, which split multi-wait matmuls to satisfy
    # the 1-sync-wait-per-instruction hardware constraint.
    nc = bacc.Bacc()

    # ---- DRAM I/O (per-core tensors; host shards batch over cores) ----
    x_d = nc.dram_tensor("x", [C, n_tok], F32, kind="ExternalInput")
    qkvw_d = nc.dram_tensor("qkv_wt", [CCH, P, 3 * C], BF16, kind="ExternalInput")
    qkbias_d = nc.dram_tensor("qk_bias", [4, P, 1], F32, kind="ExternalInput")
    vbias_d = nc.dram_tensor("v_bias", [C], F32, kind="ExternalInput")
    projw_d = nc.dram_tensor("proj_wt", [CCH, P, C], BF16, kind="ExternalInput")
    projb_d = nc.dram_tensor("proj_b", [CCH, P, 1], F32, kind="ExternalInput")
    gnsc_d = nc.dram_tensor("gn_sc", [CCH, P, 1], F32, kind="ExternalInput")
    gnbi_d = nc.dram_tensor("gn_bi", [CCH, P, 1], F32, kind="ExternalInput")
    # group-sum indicator (zero-padded to M=128 so the matmul avoids the
    # 32-column tile-mode lowering): ind[t, c, g] = (t*128 + c) // 32 == g
    gnind_d = nc.dram_tensor("gn_ind", [CCH, P, P], F32, kind="ExternalInput")
    # channel-broadcast indicator, padded to K=128: ind2[t, g, c] nonzero only g<8
    gnind2_d = nc.dram_tensor("gn_ind2", [CCH, P, P], F32, kind="ExternalInput")
    out_d = nc.dram_tensor("out", [C, n_tok], F32, kind="ExternalOutput")

    with tile.TileContext(nc) as tc:
        with (
            tc.tile_pool(name="persist", bufs=1) as pp,
            tc.tile_pool(name="work", bufs=3) as wp,
            tc.tile_pool(name="ps", bufs=3, space="PSUM") as psp,
            tc.tile_pool(name="ps_o", bufs=1, space="PSUM") as pso,
        ):
            # ---------------- load weights / constants ----------------
            qkvw = pp.tile([P, CCH, 3 * C], BF16, tag="qkvw")
            nc.sync.dma_start(qkvw[:], qkvw_d.rearrange("t p o -> p t o"))
            projw = pp.tile([P, CCH, C], BF16, tag="projw")
            nc.sync.dma_start(projw[:], projw_d.rearrange("t p o -> p t o"))
            qkb = pp.tile([P, 4], F32, tag="qkb")
            nc.sync.dma_start(qkb[:], qkbias_d.rearrange("j p one -> p (j one)"))
            projb = pp.tile([P, CCH], F32, tag="projb")
            nc.sync.dma_start(projb[:], projb_d.rearrange("t p one -> p (t one)"))
            gnsc = pp.tile([P, CCH], F32, tag="gnsc")
            nc.sync.dma_start(gnsc[:], gnsc_d.rearrange("t p one -> p (t one)"))
            gnbi = pp.tile([P, CCH], F32, tag="gnbi")
            nc.sync.dma_start(gnbi[:], gnbi_d.rearrange("t p one -> p (t one)"))
            gnind = pp.tile([P, CCH, P], F32, tag="gnind")
            nc.sync.dma_start(gnind[:], gnind_d.rearrange("t p g -> p t g"))
            gnind2 = pp.tile([P, CCH, P], F32, tag="gnind2")
            nc.sync.dma_start(gnind2[:], gnind2_d.rearrange("t g c -> g t c"))
            # V bias broadcast across partitions (DMA with partition-stride 0)
            vbias = pp.tile([P, C], F32, tag="vbias")
            nc.sync.dma_start(vbias[:], vbias_d[None, :].to_broadcast([P, C]))
            # all-ones [128, 128] used to (sum over partitions + broadcast)
            ones_mat = pp.tile([P, P], F32, tag="ones_mat")
            nc.vector.memset(ones_mat[:], 1.0)

            # ---------------- load x, GroupNorm stats ----------------
            x_sb = pp.tile([P, CCH, n_tok], F32, tag="x_sb")
            stats = pp.tile([P, CCH, 2], F32, tag="stats")
            for t in range(CCH):
                nc.sync.dma_start(x_sb[:, t], x_d[t * P:(t + 1) * P, :])
                bn6 = wp.tile([P, n_tok // 512, 6], F32, tag="bn6")
                xv = x_sb[:, t].rearrange("p (a b) -> p a b", b=512)
                for a in range(n_tok // 512):
                    nc.vector.bn_stats(bn6[:, a], xv[:, a])
                # mv = (mean, var) per partition
                nc.vector.bn_aggr(stats[:, t], bn6[:])
                # stats col1 := mean^2 + var = E[x^2] (col0 stays mean)
                nc.vector.scalar_tensor_tensor(
                    out=stats[:, t, 1:2],
                    in0=stats[:, t, 0:1],
                    scalar=stats[:, t, 0:1],
                    in1=stats[:, t, 1:2],
                    op0=ALU.mult,
                    op1=ALU.add,
                )

            # group aggregation: gagg[g, j] = sum_{c in group g} stats[c, j]
            gagg_ps = psp.tile([P, 512], F32, tag="sps", name="gagg_ps")
            for t in range(CCH):
                nc.tensor.matmul(
                    gagg_ps[:, :2],
                    gnind[:, t],
                    stats[:, t],
                    start=(t == 0),
                    stop=(t == CCH - 1),
                )
            # per-group a = rstd, b = -mean * rstd   (divide sums by GSZ first)
            gab = pp.tile([P, 2], F32, tag="gab")
            nc.vector.memset(gab[:], 0.0)
            gmean = wp.tile([P, 1], F32, tag="gmean")
            gtmp = wp.tile([P, 1], F32, tag="gtmp")
            nc.vector.tensor_scalar_mul(gmean[:GROUPS], gagg_ps[:GROUPS, 0:1], 1.0 / GSZ)
            nc.vector.tensor_scalar_mul(gtmp[:GROUPS], gagg_ps[:GROUPS, 1:2], 1.0 / GSZ)
            # gtmp := mean^2 - E[x^2] = -var
            nc.vector.scalar_tensor_tensor(
                out=gtmp[:GROUPS],
                in0=gmean[:GROUPS],
                scalar=gmean[:GROUPS],
                in1=gtmp[:GROUPS],
                op0=ALU.mult,
                op1=ALU.subtract,
            )
            # std = sqrt(-1 * gtmp + eps)
            epsb = wp.tile([P, 1], F32, tag="epsb")
            nc.vector.memset(epsb[:], EPS)
            nc.scalar.activation(gtmp[:GROUPS], gtmp[:GROUPS], AF.Sqrt,
                                 bias=epsb[:GROUPS], scale=-1.0)
            nc.vector.reciprocal(gab[:GROUPS, 0:1], gtmp[:GROUPS])  # a = rstd
            # b = -(mean * rstd):  (mean mult rstd) subtract 2*mean*rstd
            nc.vector.tensor_mul(gtmp[:GROUPS], gmean[:GROUPS], gab[:GROUPS, 0:1])
            nc.vector.tensor_scalar_mul(gab[:GROUPS, 1:2], gtmp[:GROUPS], -1.0)

            # broadcast (a, b) back to channels: chab[c, j] = gab[g(c), j]
            xn = pp.tile([P, CCH, n_tok], BF16, tag="xn")
            for t in range(CCH):
                chab_ps = psp.tile([P, 512], F32, tag="sps", name=f"chab_ps{t}")
                nc.tensor.matmul(chab_ps[:, :2], gnind2[:, t], gab[:],
                                 start=True, stop=True)
                # mult_c = a * gamma_c ; add_c = b * gamma_c + beta_c
                chm = pp.tile([P, 1], F32, tag=f"chm{t}", name=f"chm{t}")
                cha = pp.tile([P, 1], F32, tag=f"cha{t}", name=f"cha{t}")
                nc.vector.tensor_mul(chm[:], chab_ps[:, 0:1], gnsc[:, t, None])
                nc.vector.scalar_tensor_tensor(
                    out=cha[:],
                    in0=chab_ps[:, 1:2],
                    scalar=gnsc[:, t, None],
                    in1=gnbi[:, t, None],
                    op0=ALU.mult,
                    op1=ALU.add,
                )
                # xn = x * mult + add   (bf16 out)
                nc.scalar.activation(xn[:, t], x_sb[:, t], AF.Identity,
                                     bias=cha[:], scale=chm[:])

            # ---------------- QKV ----------------
            # Q, K in (d, n) layout; j = 0,1 -> Q chunks; 2,3 -> K chunks.
            # softmax scale folded into Q (scale on PSUM->SBUF copy; host
            # pre-scaled the Q bias entries).
            qk = pp.tile([P, 4, n_tok], BF16, tag="qk")
            for j in range(4):
                for ntile in range(NQ):
                    ps = psp.tile([P, 512], F32, tag="sps", name=f"qk{j}_{ntile}")
                    for t in range(CCH):
                        nc.tensor.matmul(
                            ps[:],
                            qkvw[:, t, j * P:(j + 1) * P],
                            xn[:, t, ntile * QT:(ntile + 1) * QT],
                            start=(t == 0),
                            stop=(t == CCH - 1),
                        )
                    nc.scalar.activation(
                        qk[:, j, ntile * QT:(ntile + 1) * QT], ps[:],
                        AF.Identity,
                        bias=qkb[:, j, None],
                        scale=ATT_SCALE if j < 2 else 1.0,
                    )
            # V token-major: v_sb[:, kb, d] holds V[token kb*128+p, d]
            v_sb = pp.tile([P, NKB, C], BF16, tag="v_sb")
            for kb in range(NKB):
                ps = psp.tile([P, 512], F32, tag="sps", name=f"v{kb}")
                for t in range(CCH):
                    nc.tensor.matmul(
                        ps[:, :C],
                        xn[:, t, kb * P:(kb + 1) * P],
                        qkvw[:, t, 2 * C:3 * C],
                        start=(t == 0),
                        stop=(t == CCH - 1),
                    )
                nc.vector.tensor_add(v_sb[:, kb], ps[:, :C], vbias[:])

            # ---------------- attention + proj + residual ----------------
            for qt in range(NQ):
                qs = slice(qt * QT, (qt + 1) * QT)
                o_ps0 = pso.tile([P, 512], F32, tag="o0", name=f"o0_{qt}")
                o_ps1 = pso.tile([P, 512], F32, tag="o1", name=f"o1_{qt}")
                s_acc = wp.tile([P, QT], F32, tag="s_acc", bufs=2)
                for kb in range(NKB):
                    s_ps = psp.tile([P, 512], F32, tag="sps", name=f"s_{qt}_{kb}")
                    for t in range(CCH):
                        nc.tensor.matmul(
                            s_ps[:],
                            qk[:, 2 + t, kb * P:(kb + 1) * P],
                            qk[:, t, qs],
                            start=(t == 0),
                            stop=(t == CCH - 1),
                        )
                    pt = wp.tile([P, QT], BF16, tag="pt", bufs=3)
                    nc.scalar.activation(pt[:], s_ps[:], AF.Exp)
                    if kb == 0:
                        nc.vector.tensor_copy(s_acc[:], pt[:])
                    else:
                        nc.vector.tensor_add(s_acc[:], s_acc[:], pt[:])
                    nc.tensor.matmul(o_ps0[:], v_sb[:, kb, 0:P], pt[:],
                                     start=(kb == 0), stop=(kb == NKB - 1))
                    nc.tensor.matmul(o_ps1[:], v_sb[:, kb, P:C], pt[:],
                                     start=(kb == 0), stop=(kb == NKB - 1))
                # denominators: sum over partitions of s_acc, broadcast to all
                # 128 partitions by the all-ones stationary matrix.
                den_ps = psp.tile([P, 512], F32, tag="sps", name=f"den_{qt}")
                nc.tensor.matmul(den_ps[:], ones_mat[:], s_acc[:],
                                 start=True, stop=True)
                rec = wp.tile([P, QT], F32, tag="rec", bufs=2)
                nc.vector.reciprocal(rec[:], den_ps[:])
                ob0 = wp.tile([P, QT], BF16, tag="ob0", bufs=2)
                ob1 = wp.tile([P, QT], BF16, tag="ob1", bufs=2)
                nc.vector.tensor_mul(ob0[:], o_ps0[:], rec[:])
                nc.vector.tensor_mul(ob1[:], o_ps1[:], rec[:])
                # proj + bias + residual
                for t in range(CCH):
                    p_ps = psp.tile([P, 512], F32, tag="sps", name=f"p_{qt}_{t}")
                    nc.tensor.matmul(p_ps[:], projw[:, 0, t * P:(t + 1) * P],
                                     ob0[:], start=True, stop=False)
                    nc.tensor.matmul(p_ps[:], projw[:, 1, t * P:(t + 1) * P],
                                     ob1[:], start=False, stop=True)
                    res = wp.tile([P, QT], F32, tag="res", bufs=3)
                    nc.vector.scalar_tensor_tensor(
                        out=res[:],
                        in0=p_ps[:],
                        scalar=projb[:, t, None],
                        in1=x_sb[:, t, qs],
                        op0=ALU.add,
                        op1=ALU.add,
                    )
                    nc.sync.dma_start(out_d[t * P:(t + 1) * P, qs], res[:])

    nc.finalize()
    return nc


# ---------------------------------------------------------------------------
# host side
# ---------------------------------------------------------------------------

def _prep_core_inputs(inputs, n_tok=H * W):
    """Build the per-core in_maps (shared weight tensors + per-core x)."""
    CCH = C // P
    f32 = np.float32
    bf16 = ml_dtypes.bfloat16

    x = np.asarray(inputs["x"], f32).reshape(B, C, n_tok)
    gn_scale = np.asarray(inputs["gn_scale"], f32)
    gn_bias = np.asarray(inputs["gn_bias"], f32)
    qkv_w = np.asarray(inputs["qkv_w"], f32)
    qkv_b = np.asarray(inputs["qkv_b"], f32)
    proj_w = np.asarray(inputs["proj_w"], f32)
    proj_b = np.asarray(inputs["proj_b"], f32)

    qkv_wt = np.ascontiguousarray(qkv_w.T).reshape(CCH, P, 3 * C).astype(bf16)
    qk_bias = qkv_b[:2 * C].reshape(4, P, 1).astype(f32).copy()
    qk_bias[:2] *= ATT_SCALE  # fold softmax scale into Q bias
    v_bias = qkv_b[2 * C:].astype(f32)
    proj_wt = np.ascontiguousarray(proj_w.T).reshape(CCH, P, C).astype(bf16)
    proj_bt = proj_b.reshape(CCH, P, 1).astype(f32)
    gn_sc = gn_scale.reshape(CCH, P, 1).astype(f32)
    gn_bi = gn_bias.reshape(CCH, P, 1).astype(f32)

    ch = np.arange(C)
    gn_ind = np.zeros((CCH, P, P), f32)
    gn_ind[ch // P, ch % P, ch // (C // GROUPS)] = 1.0
    gn_ind2 = np.zeros((CCH, P, P), f32)
    for t in range(CCH):
        gn_ind2[t, :GROUPS, :] = gn_ind[t, :, :GROUPS].T

    shared = {
        "qkv_wt": qkv_wt,
        "qk_bias": qk_bias,
        "v_bias": v_bias,
        "proj_wt": proj_wt,
        "proj_b": proj_bt,
        "gn_sc": gn_sc,
        "gn_bi": gn_bi,
        "gn_ind": gn_ind,
        "gn_ind2": gn_ind2,
    }
    return [dict(shared, x=np.ascontiguousarray(x[i])) for i in range(B)]


_NC_CACHE = {}
LAST_RESULT = None  # BassKernelResults of the most recent run (for test.py)


def _get_nc():
    if "nc" not in _NC_CACHE:
        _NC_CACHE["nc"] = build_nc()
    return _NC_CACHE["nc"]


def kernel(**inputs) -> np.ndarray:
    global LAST_RESULT
    from concourse.bass_utils import run_bass_kernel_spmd

    nc = _get_nc()
    in_maps = _prep_core_inputs(inputs)
    res = run_bass_kernel_spmd(nc, in_maps, list(range(N_CORES)))
    LAST_RESULT = res
    out = np.stack([np.asarray(res.results[i]["out"]) for i in range(B)])
    return out.reshape(B, C, H, W).astype(np.float32)


# revision 15
# speedup vs baseline: 1.1293x; 1.1293x over previous
"""Trainium2 Bass kernel for nn_AttentionBlock (GroupNorm + single-head
self-attention + proj + residual), data-parallel over batch on 8 cores.

Contract: kernel(**inputs) takes the FULL unsharded inputs
  x (8, 256, 64, 64) f32, gn_scale (256,), gn_bias (256,),
  qkv_w (768, 256), qkv_b (768,), proj_w (256, 256), proj_b (256,)
and returns the FULL output (8, 256, 64, 64) f32.

Per-core plan (one sample per core):
  - x viewed as (C=256, N=4096) = (channels on partitions, tokens on free dim)
  - GroupNorm(8 groups) stats via bn_stats/bn_aggr + tiny indicator matmuls
  - xn cast to bf16; QKV as channel matmuls:
      Q, K produced in (d, n) layout  [d on partitions]
      V produced in token-major (n, d) layout
    so that attention needs NO transposes:
      scoresT[k, q] = sum_d K[d,k] Q[d,q]   (lhsT=K tile, rhs=Q tile)
      PT = exp(scoresT)                     (softmax scale folded into Q)
      out_un[d, q] = sum_k V[k,d] PT[k,q]   (lhsT=V tile, rhs=PT)
      denom[q] = sum_k PT[k,q]  -> DVE accumulation + one ones-matmul that
                                   also broadcasts the sum over partitions
  - out = proj(out_un * 1/denom) + proj_b + x  (residual in f32)
"""

import os
import sys

import numpy as np

for _p in (
    "/opt/trn_rl_repo",
    "/root/.axon_site",
    "/root/.axon_site/_ro/trn_rl_repo",
    "/root/.axon_site/_ro/pypackages",
):
    if os.path.isdir(_p) and _p not in sys.path:
        sys.path.append(_p)

import ml_dtypes  # noqa: E402

import concourse.bass as bass  # noqa: E402
import concourse.mybir as mybir  # noqa: E402
import concourse.tile as tile  # noqa: E402
from concourse import bacc  # noqa: E402

F32 = mybir.dt.float32
BF16 = mybir.dt.bfloat16
AF = mybir.ActivationFunctionType
ALU = mybir.AluOpType

B, C, H, W = 8, 256, 64, 64
GROUPS = 8
EPS = 1e-5
P = 128
N_CORES = 8
ATT_SCALE = float(C) ** -0.5  # 1/16


def build_nc(n_tok=H * W):
    """Build the single-core Bass program (SPMD across 8 cores)."""
    CCH = C // P            # channel chunks (2)
    QT = 512                # q-tile width (one PSUM bank of f32)
    NQ = n_tok // QT        # number of q tiles
    NKB = n_tok // P        # number of 128-token key blocks
    GSZ = C // GROUPS       # channels per group (32)
    G_PER_CHUNK = GROUPS // CCH  # groups per 128-channel chunk (4)

    # Bacc (not plain Bass): its compile() runs move_matmul_waits_to_ldweights
    # + generate_event_semaphores, which split multi-wait matmuls to satisfy
    # the 1-sync-wait-per-instruction hardware constraint.
    nc = bacc.Bacc()

    # ---- DRAM I/O (per-core tensors; host shards batch over cores) ----
    x_d = nc.dram_tensor("x", [C, n_tok], F32, kind="ExternalInput")
    qkvw_d = nc.dram_tensor("qkv_wt", [CCH, P, 3 * C], BF16, kind="ExternalInput")
    qkbias_d = nc.dram_tensor("qk_bias", [4, P, 1], F32, kind="ExternalInput")
    vbias_d = nc.dram_tensor("v_bias", [C], F32, kind="ExternalInput")
    projw_d = nc.dram_tensor("proj_wt", [CCH, P, C], BF16, kind="ExternalInput")
    projb_d = nc.dram_tensor("proj_b", [CCH, P, 1], F32, kind="ExternalInput")
    gnsc_d = nc.dram_tensor("gn_sc", [CCH, P, 1], F32, kind="ExternalInput")
    gnbi_d = nc.dram_tensor("gn_bi", [CCH, P, 1], F32, kind="ExternalInput")
    # group-sum indicator (zero-padded to M=128 so the matmul avoids the
    # 32-column tile-mode lowering): ind[t, c, g] = (t*128 + c) // 32 == g
    gnind_d = nc.dram_tensor("gn_ind", [CCH, P, P], F32, kind="ExternalInput")
    # channel-broadcast indicator, padded to K=128: ind2[t, g, c] nonzero only g<8
    gnind2_d = nc.dram_tensor("gn_ind2", [CCH, P, P], F32, kind="ExternalInput")
    out_d = nc.dram_tensor("out", [C, n_tok], F32, kind="ExternalOutput")

    QP = 2 * QT             # paired q-tile width (1024)
    NQP = n_tok // QP       # number of q-tile pairs

    with tile.TileContext(nc) as tc:
        with (
            tc.tile_pool(name="persist", bufs=1) as pp,
            tc.tile_pool(name="work", bufs=3) as wp,
            tc.tile_pool(name="ps", bufs=1, space="PSUM") as psp,
            tc.tile_pool(name="ps_sc", bufs=3, space="PSUM") as psb,
            tc.tile_pool(name="ps_o", bufs=1, space="PSUM") as pso,
        ):
            # ---------------- load weights / constants ----------------
            qkvw = pp.tile([P, CCH, 3 * C], BF16, tag="qkvw")
            nc.sync.dma_start(qkvw[:], qkvw_d.rearrange("t p o -> p t o"))
            projw = pp.tile([P, CCH, C], BF16, tag="projw")
            nc.sync.dma_start(projw[:], projw_d.rearrange("t p o -> p t o"))
            qkb = pp.tile([P, 4], F32, tag="qkb")
            nc.sync.dma_start(qkb[:], qkbias_d.rearrange("j p one -> p (j one)"))
            projb = pp.tile([P, CCH], F32, tag="projb")
            nc.sync.dma_start(projb[:], projb_d.rearrange("t p one -> p (t one)"))
            gnsc = pp.tile([P, CCH], F32, tag="gnsc")
            nc.sync.dma_start(gnsc[:], gnsc_d.rearrange("t p one -> p (t one)"))
            gnbi = pp.tile([P, CCH], F32, tag="gnbi")
            nc.sync.dma_start(gnbi[:], gnbi_d.rearrange("t p one -> p (t one)"))
            gnind = pp.tile([P, CCH, P], F32, tag="gnind")
            nc.sync.dma_start(gnind[:], gnind_d.rearrange("t p g -> p t g"))
            gnind2 = pp.tile([P, CCH, P], F32, tag="gnind2")
            nc.sync.dma_start(gnind2[:], gnind2_d.rearrange("t g c -> g t c"))
            # V bias broadcast across partitions (DMA with partition-stride 0)
            vbias = pp.tile([P, C], F32, tag="vbias")
            nc.sync.dma_start(vbias[:], vbias_d[None, :].to_broadcast([P, C]))
            # all-ones [128, 128] used to (sum over partitions + broadcast)
            ones_mat = pp.tile([P, P], F32, tag="ones_mat")
            nc.vector.memset(ones_mat[:], 1.0)

            # ---------------- load x, GroupNorm stats ----------------
            x_sb = pp.tile([P, CCH, n_tok], F32, tag="x_sb")
            stats = pp.tile([P, CCH, 2], F32, tag="stats")
            for t in range(CCH):
                nc.sync.dma_start(x_sb[:, t], x_d[t * P:(t + 1) * P, :])
                bn6 = wp.tile([P, n_tok // 512, 6], F32, tag="bn6")
                xv = x_sb[:, t].rearrange("p (a b) -> p a b", b=512)
                for a in range(n_tok // 512):
                    nc.vector.bn_stats(bn6[:, a], xv[:, a])
                # mv = (mean, var) per partition
                nc.vector.bn_aggr(stats[:, t], bn6[:])
                # stats col1 := mean^2 + var = E[x^2] (col0 stays mean)
                nc.vector.scalar_tensor_tensor(
                    out=stats[:, t, 1:2],
                    in0=stats[:, t, 0:1],
                    scalar=stats[:, t, 0:1],
                    in1=stats[:, t, 1:2],
                    op0=ALU.mult,
                    op1=ALU.add,
                )

            # group aggregation: gagg[g, j] = sum_{c in group g} stats[c, j]
            gagg_ps = psp.tile([P, 512], F32, tag="sps", name="gagg_ps")
            for t in range(CCH):
                nc.tensor.matmul(
                    gagg_ps[:, :2],
                    gnind[:, t],
                    stats[:, t],
                    start=(t == 0),
                    stop=(t == CCH - 1),
                )
            # per-group a = rstd, b = -mean * rstd   (divide sums by GSZ first)
            gab = pp.tile([P, 2], F32, tag="gab")
            nc.vector.memset(gab[:], 0.0)
            gmean = wp.tile([P, 1], F32, tag="gmean")
            gtmp = wp.tile([P, 1], F32, tag="gtmp")
            nc.vector.tensor_scalar_mul(gmean[:GROUPS], gagg_ps[:GROUPS, 0:1], 1.0 / GSZ)
            nc.vector.tensor_scalar_mul(gtmp[:GROUPS], gagg_ps[:GROUPS, 1:2], 1.0 / GSZ)
            # gtmp := mean^2 - E[x^2] = -var
            nc.vector.scalar_tensor_tensor(
                out=gtmp[:GROUPS],
                in0=gmean[:GROUPS],
                scalar=gmean[:GROUPS],
                in1=gtmp[:GROUPS],
                op0=ALU.mult,
                op1=ALU.subtract,
            )
            # std = sqrt(-1 * gtmp + eps)
            epsb = wp.tile([P, 1], F32, tag="epsb")
            nc.vector.memset(epsb[:], EPS)
            nc.scalar.activation(gtmp[:GROUPS], gtmp[:GROUPS], AF.Sqrt,
                                 bias=epsb[:GROUPS], scale=-1.0)
            nc.vector.reciprocal(gab[:GROUPS, 0:1], gtmp[:GROUPS])  # a = rstd
            # b = -(mean * rstd):  (mean mult rstd) subtract 2*mean*rstd
            nc.vector.tensor_mul(gtmp[:GROUPS], gmean[:GROUPS], gab[:GROUPS, 0:1])
            nc.vector.tensor_scalar_mul(gab[:GROUPS, 1:2], gtmp[:GROUPS], -1.0)

            # broadcast (a, b) back to channels: chab[c, j] = gab[g(c), j]
            xn = pp.tile([P, CCH, n_tok], BF16, tag="xn")
            for t in range(CCH):
                chab_ps = psp.tile([P, 512], F32, tag="sps", name=f"chab_ps{t}")
                nc.tensor.matmul(chab_ps[:, :2], gnind2[:, t], gab[:],
                                 start=True, stop=True)
                # mult_c = a * gamma_c ; add_c = b * gamma_c + beta_c
                chm = pp.tile([P, 1], F32, tag=f"chm{t}", name=f"chm{t}")
                cha = pp.tile([P, 1], F32, tag=f"cha{t}", name=f"cha{t}")
                nc.vector.tensor_mul(chm[:], chab_ps[:, 0:1], gnsc[:, t, None])
                nc.vector.scalar_tensor_tensor(
                    out=cha[:],
                    in0=chab_ps[:, 1:2],
                    scalar=gnsc[:, t, None],
                    in1=gnbi[:, t, None],
                    op0=ALU.mult,
                    op1=ALU.add,
                )
                # xn = x * mult + add   (bf16 out)
                nc.scalar.activation(xn[:, t], x_sb[:, t], AF.Identity,
                                     bias=cha[:], scale=chm[:])

            # ---------------- QKV ----------------
            # Q, K in (d, n) layout; j = 0,1 -> Q chunks; 2,3 -> K chunks.
            # softmax scale folded into Q (scale on PSUM->SBUF copy; host
            # pre-scaled the Q bias entries). N=1024 matmuls with bf16 PSUM
            # halve the matmul/copy instruction counts.
            qk = pp.tile([P, 4, n_tok], BF16, tag="qk")
            for j in range(4):
                for ntile in range(NQ):
                    ns = slice(ntile * QT, (ntile + 1) * QT)
                    ps = psb.tile([P, 512], F32, tag="sc", name=f"qk{j}_{ntile}")
                    for t in range(CCH):
                        nc.tensor.matmul(
                            ps[:],
                            qkvw[:, t, j * P:(j + 1) * P],
                            xn[:, t, ns],
                            start=(t == 0),
                            stop=(t == CCH - 1),
                        )
                    nc.scalar.activation(
                        qk[:, j, ns], ps[:],
                        AF.Identity,
                        bias=qkb[:, j, None],
                        scale=ATT_SCALE if j < 2 else 1.0,
                    )
            # V token-major: v_sb[:, kb, d] holds V[token kb*128+p, d]
            v_sb = pp.tile([P, NKB, C], BF16, tag="v_sb")
            for kb in range(NKB):
                ps = psp.tile([P, 512], F32, tag="sps", name=f"v{kb}")
                for t in range(CCH):
                    nc.tensor.matmul(
                        ps[:, :C],
                        xn[:, t, kb * P:(kb + 1) * P],
                        qkvw[:, t, 2 * C:3 * C],
                        start=(t == 0),
                        stop=(t == CCH - 1),
                    )
                nc.vector.tensor_add(v_sb[:, kb], ps[:, :C], vbias[:])

            # ---------------- attention + proj + residual ----------------
            # Processed in q-tile PAIRS (1024 q columns): scores/exp/s_acc
            # run pair-wide; PV stays per-qt (PSUM fp32 accumulation).
            # Each pair's finalize (denominator, normalize, proj, residual)
            # is DEFERRED into the next pair's kb loop so the PE never
            # stalls on the DVE chain at pair boundaries.
            def finalize_pair(st):
                pr, o_ps, s_acc = st
                rec = wp.tile([P, QP], F32, tag="rec", bufs=2)
                obs = []
                for qi in range(2):
                    qh = slice(qi * QT, (qi + 1) * QT)
                    den = psp.tile([P, 512], F32, tag="sps",
                                   name=f"den_{pr}_{qi}")
                    nc.tensor.matmul(den[:], ones_mat[:], s_acc[:, qh],
                                     start=True, stop=True)
                    nc.vector.reciprocal_approx_fast(rec[:, qh], den[:])
                for ch in range(2):
                    ob = wp.tile([P, QP], BF16, tag=f"ob{ch}", bufs=2,
                                 name=f"ob{ch}_{pr}")
                    for qi in range(2):
                        qh = slice(qi * QT, (qi + 1) * QT)
                        nc.vector.tensor_mul(ob[:, qh], o_ps[qi][ch][:],
                                             rec[:, qh])
                    obs.append(ob)
                for t in range(CCH):
                    for qi in range(2):
                        qh = slice(qi * QT, (qi + 1) * QT)
                        qg = slice(pr * QP + qi * QT, pr * QP + (qi + 1) * QT)
                        p_ps = psb.tile([P, 512], F32, tag="sc",
                                        name=f"p_{pr}_{t}_{qi}")
                        nc.tensor.matmul(p_ps[:],
                                         projw[:, 0, t * P:(t + 1) * P],
                                         obs[0][:, qh], start=True, stop=False)
                        nc.tensor.matmul(p_ps[:],
                                         projw[:, 1, t * P:(t + 1) * P],
                                         obs[1][:, qh], start=False, stop=True)
                        res = wp.tile([P, QT], F32, tag="res", bufs=3)
                        nc.vector.scalar_tensor_tensor(
                            out=res[:],
                            in0=p_ps[:],
                            scalar=projb[:, t, None],
                            in1=x_sb[:, t, qg],
                            op0=ALU.add,
                            op1=ALU.add,
                        )
                        nc.sync.dma_start(out_d[t * P:(t + 1) * P, qg], res[:])

            pending = None
            for pr in range(NQP):
                qps = slice(pr * QP, (pr + 1) * QP)
                o_ps = [[pso.tile([P, 512], F32, tag=f"o{qi}{ch}",
                                  name=f"o{qi}{ch}_{pr}")
                         for ch in range(2)] for qi in range(2)]
                s_acc = wp.tile([P, QP], F32, tag="s_acc", bufs=2)
                for kb in range(NKB):
                    pt = wp.tile([P, QP], BF16, tag="pt", bufs=3)
                    for qi in range(2):
                        qh = slice(pr * QP + qi * QT, pr * QP + (qi + 1) * QT)
                        s_ps = psb.tile([P, 512], F32, tag="sc",
                                        name=f"s_{pr}_{kb}_{qi}")
                        for t in range(CCH):
                            nc.tensor.matmul(
                                s_ps[:],
                                qk[:, 2 + t, kb * P:(kb + 1) * P],
                                qk[:, t, qh],
                                start=(t == 0),
                                stop=(t == CCH - 1),
                            )
                        nc.scalar.activation(pt[:, qi * QT:(qi + 1) * QT],
                                             s_ps[:], AF.Exp)
                    if kb == 0:
                        nc.vector.tensor_copy(s_acc[:], pt[:])
                    else:
                        nc.vector.tensor_add(s_acc[:], s_acc[:], pt[:])
                    for ch in range(2):
                        vt = v_sb[:, kb, ch * P:(ch + 1) * P]
                        for qi in range(2):
                            nc.tensor.matmul(
                                o_ps[qi][ch][:], vt,
                                pt[:, qi * QT:(qi + 1) * QT],
                                start=(kb == 0), stop=(kb == NKB - 1))
                    if kb == 2 and pending is not None:
                        finalize_pair(pending)
                        pending = None
                pending = (pr, o_ps, s_acc)
            finalize_pair(pending)

    nc.finalize()
    return nc


# ---------------------------------------------------------------------------
# host side
# ---------------------------------------------------------------------------

def _prep_core_inputs(inputs, n_tok=H * W):
    """Build the per-core in_maps (shared weight tensors + per-core x)."""
    CCH = C // P
    f32 = np.float32
    bf16 = ml_dtypes.bfloat16

    x = np.asarray(inputs["x"], f32).reshape(B, C, n_tok)
    gn_scale = np.asarray(inputs["gn_scale"], f32)
    gn_bias = np.asarray(inputs["gn_bias"], f32)
    qkv_w = np.asarray(inputs["qkv_w"], f32)
    qkv_b = np.asarray(inputs["qkv_b"], f32)
    proj_w = np.asarray(inputs["proj_w"], f32)
    proj_b = np.asarray(inputs["proj_b"], f32)

    qkv_wt = np.ascontiguousarray(qkv_w.T).reshape(CCH, P, 3 * C).astype(bf16)
    qk_bias = qkv_b[:2 * C].reshape(4, P, 1).astype(f32).copy()
    qk_bias[:2] *= ATT_SCALE  # fold softmax scale into Q bias
    v_bias = qkv_b[2 * C:].astype(f32)
    proj_wt = np.ascontiguousarray(proj_w.T).reshape(CCH, P, C).astype(bf16)
    proj_bt = proj_b.reshape(CCH, P, 1).astype(f32)
    gn_sc = gn_scale.reshape(CCH, P, 1).astype(f32)
    gn_bi = gn_bias.reshape(CCH, P, 1).astype(f32)

    ch = np.arange(C)
    gn_ind = np.zeros((CCH, P, P), f32)
    gn_ind[ch // P, ch % P, ch // (C // GROUPS)] = 1.0
    gn_ind2 = np.zeros((CCH, P, P), f32)
    for t in range(CCH):
        gn_ind2[t, :GROUPS, :] = gn_ind[t, :, :GROUPS].T

    shared = {
        "qkv_wt": qkv_wt,
        "qk_bias": qk_bias,
        "v_bias": v_bias,
        "proj_wt": proj_wt,
        "proj_b": proj_bt,
        "gn_sc": gn_sc,
        "gn_bi": gn_bi,
        "gn_ind": gn_ind,
        "gn_ind2": gn_ind2,
    }
    return [dict(shared, x=np.ascontiguousarray(x[i])) for i in range(B)]


_NC_CACHE = {}
LAST_RESULT = None  # BassKernelResults of the most recent run (for test.py)


def _get_nc():
    if "nc" not in _NC_CACHE:
        _NC_CACHE["nc"] = build_nc()
    return _NC_CACHE["nc"]


def kernel(**inputs) -> np.ndarray:
    global LAST_RESULT
    from concourse.bass_utils import run_bass_kernel_spmd

    nc = _get_nc()
    in_maps = _prep_core_inputs(inputs)
    res = run_bass_kernel_spmd(nc, in_maps, list(range(N_CORES)))
    LAST_RESULT = res
    out = np.stack([np.asarray(res.results[i]["out"]) for i in range(B)])
    return out.reshape(B, C, H, W).astype(np.float32)


# revision 19
# speedup vs baseline: 1.3225x; 1.1711x over previous
"""Trainium2 Bass kernel for nn_AttentionBlock (GroupNorm + single-head
self-attention + proj + residual), data-parallel over batch on 8 cores.

Contract: kernel(**inputs) takes the FULL unsharded inputs
  x (8, 256, 64, 64) f32, gn_scale (256,), gn_bias (256,),
  qkv_w (768, 256), qkv_b (768,), proj_w (256, 256), proj_b (256,)
and returns the FULL output (8, 256, 64, 64) f32.

Per-core plan (one sample per core):
  - x viewed as (C=256, N=4096) = (channels on partitions, tokens on free dim)
  - GroupNorm(8 groups) stats via bn_stats/bn_aggr + tiny indicator matmuls
  - xn cast to bf16; QKV as channel matmuls:
      Q, K produced in (d, n) layout  [d on partitions]
      V produced in token-major (n, d) layout
    so that attention needs NO transposes:
      scoresT[k, q] = sum_d K[d,k] Q[d,q]   (lhsT=K tile, rhs=Q tile)
      PT = exp(scoresT)                     (softmax scale folded into Q)
      out_un[d, q] = sum_k V[k,d] PT[k,q]   (lhsT=V tile, rhs=PT)
      denom[q] = sum_k PT[k,q]  -> DVE accumulation + one ones-matmul that
                                   also broadcasts the sum over partitions
  - out = proj(out_un * 1/denom) + proj_b + x  (residual in f32)
"""

import os
import sys

import numpy as np

for _p in (
    "/opt/trn_rl_repo",
    "/root/.axon_site",
    "/root/.axon_site/_ro/trn_rl_repo",
    "/root/.axon_site/_ro/pypackages",
):
    if os.path.isdir(_p) and _p not in sys.path:
        sys.path.append(_p)

import ml_dtypes  # noqa: E402

import concourse.bass as bass  # noqa: E402
import concourse.mybir as mybir  # noqa: E402
import concourse.tile as tile  # noqa: E402
from concourse import bacc  # noqa: E402

F32 = mybir.dt.float32
BF16 = mybir.dt.bfloat16
FP8 = mybir.dt.float8e4
AF = mybir.ActivationFunctionType
ALU = mybir.AluOpType
DR = mybir.MatmulPerfMode.DoubleRow

B, C, H, W = 8, 256, 64, 64
GROUPS = 8
EPS = 1e-5
P = 128
N_CORES = 8
ATT_SCALE = float(C) ** -0.5  # 1/16


def build_nc(n_tok=H * W):
    """Build the single-core Bass program (SPMD across 8 cores)."""
    CCH = C // P            # channel chunks (2)
    QT = 512                # q-tile width (one PSUM bank of f32)
    NQ = n_tok // QT        # number of q tiles
    NKB = n_tok // P        # number of 128-token key blocks
    GSZ = C // GROUPS       # channels per group (32)
    G_PER_CHUNK = GROUPS // CCH  # groups per 128-channel chunk (4)

    # Bacc (not plain Bass): its compile() runs move_matmul_waits_to_ldweights
    # + generate_event_semaphores, which split multi-wait matmuls to satisfy
    # the 1-sync-wait-per-instruction hardware constraint.
    nc = bacc.Bacc()

    # ---- DRAM I/O (per-core tensors; host shards batch over cores) ----
    x_d = nc.dram_tensor("x", [C, n_tok], F32, kind="ExternalInput")
    qkvw_d = nc.dram_tensor("qkv_wt", [CCH, P, 3 * C], BF16, kind="ExternalInput")
    qkbias_d = nc.dram_tensor("qk_bias", [4, P, 1], F32, kind="ExternalInput")
    vbias_d = nc.dram_tensor("v_bias", [C], F32, kind="ExternalInput")
    projw_d = nc.dram_tensor("proj_wt", [CCH, P, C], BF16, kind="ExternalInput")
    projb_d = nc.dram_tensor("proj_b", [CCH, P, 1], F32, kind="ExternalInput")
    gnsc_d = nc.dram_tensor("gn_sc", [CCH, P, 1], F32, kind="ExternalInput")
    gnbi_d = nc.dram_tensor("gn_bi", [CCH, P, 1], F32, kind="ExternalInput")
    # group-sum indicator (zero-padded to M=128 so the matmul avoids the
    # 32-column tile-mode lowering): ind[t, c, g] = (t*128 + c) // 32 == g
    gnind_d = nc.dram_tensor("gn_ind", [CCH, P, P], F32, kind="ExternalInput")
    # channel-broadcast indicator, padded to K=128: ind2[t, g, c] nonzero only g<8
    gnind2_d = nc.dram_tensor("gn_ind2", [CCH, P, P], F32, kind="ExternalInput")
    out_d = nc.dram_tensor("out", [C, n_tok], F32, kind="ExternalOutput")

    QP = 2 * QT             # paired q-tile width (1024)
    NQP = n_tok // QP       # number of q-tile pairs

    with tile.TileContext(nc) as tc:
        with (
            tc.tile_pool(name="persist", bufs=1) as pp,
            tc.tile_pool(name="work", bufs=3) as wp,
            tc.tile_pool(name="ps", bufs=1, space="PSUM") as psp,
            tc.tile_pool(name="ps_sc", bufs=3, space="PSUM") as psb,
            tc.tile_pool(name="ps_o", bufs=1, space="PSUM") as pso,
        ):
            # ---------------- load weights / constants ----------------
            qkvw = pp.tile([P, CCH, 3 * C], BF16, tag="qkvw")
            nc.sync.dma_start(qkvw[:], qkvw_d.rearrange("t p o -> p t o"))
            projw = pp.tile([P, CCH, C], BF16, tag="projw")
            nc.sync.dma_start(projw[:], projw_d.rearrange("t p o -> p t o"))
            qkb = pp.tile([P, 4], F32, tag="qkb")
            nc.sync.dma_start(qkb[:], qkbias_d.rearrange("j p one -> p (j one)"))
            projb = pp.tile([P, CCH], F32, tag="projb")
            nc.sync.dma_start(projb[:], projb_d.rearrange("t p one -> p (t one)"))
            gnsc = pp.tile([P, CCH], F32, tag="gnsc")
            nc.sync.dma_start(gnsc[:], gnsc_d.rearrange("t p one -> p (t one)"))
            gnbi = pp.tile([P, CCH], F32, tag="gnbi")
            nc.sync.dma_start(gnbi[:], gnbi_d.rearrange("t p one -> p (t one)"))
            gnind = pp.tile([P, CCH, P], F32, tag="gnind")
            nc.sync.dma_start(gnind[:], gnind_d.rearrange("t p g -> p t g"))
            gnind2 = pp.tile([P, CCH, P], F32, tag="gnind2")
            nc.sync.dma_start(gnind2[:], gnind2_d.rearrange("t g c -> g t c"))
            # V bias broadcast across partitions (DMA with partition-stride 0)
            vbias = pp.tile([P, C], F32, tag="vbias")
            nc.sync.dma_start(vbias[:], vbias_d[None, :].to_broadcast([P, C]))
            # all-ones [128, 128] used to (sum over partitions + broadcast)
            ones_mat = pp.tile([P, P], F32, tag="ones_mat")
            nc.vector.memset(ones_mat[:], 1.0)

            # ---------------- load x, GroupNorm stats ----------------
            x_sb = pp.tile([P, CCH, n_tok], F32, tag="x_sb")
            stats = pp.tile([P, CCH, 2], F32, tag="stats")
            for t in range(CCH):
                nc.sync.dma_start(x_sb[:, t], x_d[t * P:(t + 1) * P, :])
                bn6 = wp.tile([P, n_tok // 512, 6], F32, tag="bn6")
                xv = x_sb[:, t].rearrange("p (a b) -> p a b", b=512)
                for a in range(n_tok // 512):
                    nc.vector.bn_stats(bn6[:, a], xv[:, a])
                # mv = (mean, var) per partition
                nc.vector.bn_aggr(stats[:, t], bn6[:])
                # stats col1 := mean^2 + var = E[x^2] (col0 stays mean)
                nc.vector.scalar_tensor_tensor(
                    out=stats[:, t, 1:2],
                    in0=stats[:, t, 0:1],
                    scalar=stats[:, t, 0:1],
                    in1=stats[:, t, 1:2],
                    op0=ALU.mult,
                    op1=ALU.add,
                )

            # group aggregation: gagg[g, j] = sum_{c in group g} stats[c, j]
            gagg_ps = psp.tile([P, 512], F32, tag="sps", name="gagg_ps")
            for t in range(CCH):
                nc.tensor.matmul(
                    gagg_ps[:, :2],
                    gnind[:, t],
                    stats[:, t],
                    start=(t == 0),
                    stop=(t == CCH - 1),
                )
            # per-group a = rstd, b = -mean * rstd   (divide sums by GSZ first)
            gab = pp.tile([P, 2], F32, tag="gab")
            nc.vector.memset(gab[:], 0.0)
            gmean = wp.tile([P, 1], F32, tag="gmean")
            gtmp = wp.tile([P, 1], F32, tag="gtmp")
            nc.vector.tensor_scalar_mul(gmean[:GROUPS], gagg_ps[:GROUPS, 0:1], 1.0 / GSZ)
            nc.vector.tensor_scalar_mul(gtmp[:GROUPS], gagg_ps[:GROUPS, 1:2], 1.0 / GSZ)
            # gtmp := mean^2 - E[x^2] = -var
            nc.vector.scalar_tensor_tensor(
                out=gtmp[:GROUPS],
                in0=gmean[:GROUPS],
                scalar=gmean[:GROUPS],
                in1=gtmp[:GROUPS],
                op0=ALU.mult,
                op1=ALU.subtract,
            )
            # std = sqrt(-1 * gtmp + eps)
            epsb = wp.tile([P, 1], F32, tag="epsb")
            nc.vector.memset(epsb[:], EPS)
            nc.scalar.activation(gtmp[:GROUPS], gtmp[:GROUPS], AF.Sqrt,
                                 bias=epsb[:GROUPS], scale=-1.0)
            nc.vector.reciprocal(gab[:GROUPS, 0:1], gtmp[:GROUPS])  # a = rstd
            # b = -(mean * rstd):  (mean mult rstd) subtract 2*mean*rstd
            nc.vector.tensor_mul(gtmp[:GROUPS], gmean[:GROUPS], gab[:GROUPS, 0:1])
            nc.vector.tensor_scalar_mul(gab[:GROUPS, 1:2], gtmp[:GROUPS], -1.0)

            # broadcast (a, b) back to channels: chab[c, j] = gab[g(c), j]
            xn = pp.tile([P, CCH, n_tok], BF16, tag="xn")
            for t in range(CCH):
                chab_ps = psp.tile([P, 512], F32, tag="sps", name=f"chab_ps{t}")
                nc.tensor.matmul(chab_ps[:, :2], gnind2[:, t], gab[:],
                                 start=True, stop=True)
                # mult_c = a * gamma_c ; add_c = b * gamma_c + beta_c
                chm = pp.tile([P, 1], F32, tag=f"chm{t}", name=f"chm{t}")
                cha = pp.tile([P, 1], F32, tag=f"cha{t}", name=f"cha{t}")
                nc.vector.tensor_mul(chm[:], chab_ps[:, 0:1], gnsc[:, t, None])
                nc.vector.scalar_tensor_tensor(
                    out=cha[:],
                    in0=chab_ps[:, 1:2],
                    scalar=gnsc[:, t, None],
                    in1=gnbi[:, t, None],
                    op0=ALU.mult,
                    op1=ALU.add,
                )
                # xn = x * mult + add   (bf16 out)
                nc.scalar.activation(xn[:, t], x_sb[:, t], AF.Identity,
                                     bias=cha[:], scale=chm[:])

            # ---------------- QKV ----------------
            # Q, K in (d, n) layout; j = 0,1 -> Q chunks; 2,3 -> K chunks.
            # softmax scale folded into Q (scale on PSUM->SBUF copy; host
            # pre-scaled the Q bias entries). N=1024 matmuls with bf16 PSUM
            # halve the matmul/copy instruction counts.
            # Q, K, V stored in fp8e4 (e4m3) for DoubleRow matmuls. The
            # softmax scale is NOT folded into Q here (it would push |q| to
            # ~0.02, into fp8 subnormals) — it moves into the exp() scale.
            qk = pp.tile([P, 4, n_tok], FP8, tag="qk")
            for j in range(4):
                for ntile in range(NQ):
                    ns = slice(ntile * QT, (ntile + 1) * QT)
                    ps = psb.tile([P, 512], F32, tag="sc", name=f"qk{j}_{ntile}")
                    for t in range(CCH):
                        nc.tensor.matmul(
                            ps[:],
                            qkvw[:, t, j * P:(j + 1) * P],
                            xn[:, t, ns],
                            start=(t == 0),
                            stop=(t == CCH - 1),
                        )
                    nc.scalar.activation(
                        qk[:, j, ns], ps[:],
                        AF.Identity,
                        bias=qkb[:, j, None],
                        scale=1.0,
                    )
            # V token-major: v_sb[:, kb, d] holds V[token kb*128+p, d]
            v_sb = pp.tile([P, NKB, C], FP8, tag="v_sb")
            for kb in range(NKB):
                ps = psp.tile([P, 512], F32, tag="sps", name=f"v{kb}")
                for t in range(CCH):
                    nc.tensor.matmul(
                        ps[:, :C],
                        xn[:, t, kb * P:(kb + 1) * P],
                        qkvw[:, t, 2 * C:3 * C],
                        start=(t == 0),
                        stop=(t == CCH - 1),
                    )
                nc.vector.tensor_add(v_sb[:, kb], ps[:, :C], vbias[:])

            # ---------------- attention + proj + residual ----------------
            # Processed in q-tile PAIRS (1024 q columns): scores/exp/s_acc
            # run pair-wide; PV stays per-qt (PSUM fp32 accumulation).
            # Each pair's finalize (denominator, normalize, proj, residual)
            # is DEFERRED into the next pair's kb loop so the PE never
            # stalls on the DVE chain at pair boundaries.
            def finalize_pair(st):
                pr, o_ps, s_acc = st
                rec = wp.tile([P, QP], F32, tag="rec", bufs=2)
                obs = []
                for qi in range(2):
                    qh = slice(qi * QT, (qi + 1) * QT)
                    den = psp.tile([P, 512], F32, tag="sps",
                                   name=f"den_{pr}_{qi}")
                    nc.tensor.matmul(den[:], ones_mat[:], s_acc[:, qh],
                                     start=True, stop=True)
                    nc.vector.reciprocal_approx_fast(rec[:, qh], den[:])
                for ch in range(2):
                    ob = wp.tile([P, QP], BF16, tag=f"ob{ch}", bufs=2,
                                 name=f"ob{ch}_{pr}")
                    for qi in range(2):
                        qh = slice(qi * QT, (qi + 1) * QT)
                        nc.vector.tensor_mul(ob[:, qh], o_ps[qi][ch][:],
                                             rec[:, qh])
                    obs.append(ob)
                for t in range(CCH):
                    for qi in range(2):
                        qh = slice(qi * QT, (qi + 1) * QT)
                        qg = slice(pr * QP + qi * QT, pr * QP + (qi + 1) * QT)
                        p_ps = psb.tile([P, 512], F32, tag="sc",
                                        name=f"p_{pr}_{t}_{qi}")
                        nc.tensor.matmul(p_ps[:],
                                         projw[:, 0, t * P:(t + 1) * P],
                                         obs[0][:, qh], start=True, stop=False)
                        nc.tensor.matmul(p_ps[:],
                                         projw[:, 1, t * P:(t + 1) * P],
                                         obs[1][:, qh], start=False, stop=True)
                        res = wp.tile([P, QT], F32, tag="res", bufs=3)
                        nc.vector.scalar_tensor_tensor(
                            out=res[:],
                            in0=p_ps[:],
                            scalar=projb[:, t, None],
                            in1=x_sb[:, t, qg],
                            op0=ALU.add,
                            op1=ALU.add,
                        )
                        nc.sync.dma_start(out_d[t * P:(t + 1) * P, qg], res[:])

            pending = None
            for pr in range(NQP):
                qps = slice(pr * QP, (pr + 1) * QP)
                o_ps = [[pso.tile([P, 512], F32, tag=f"o{qi}{ch}",
                                  name=f"o{qi}{ch}_{pr}")
                         for ch in range(2)] for qi in range(2)]
                s_acc = wp.tile([P, QP], F32, tag="s_acc", bufs=2)
                for kbp in range(NKB // 2):
                    # pt holds exp(scores) for the TWO key blocks of this
                    # DoubleRow pair: plane i = key block kbp*2+i (fp8).
                    pt = wp.tile([P, 2, QP], FP8, tag="pt", bufs=3)
                    for k2 in range(2):
                        kb = 2 * kbp + k2
                        for qi in range(2):
                            qh = slice(pr * QP + qi * QT,
                                       pr * QP + (qi + 1) * QT)
                            s_ps = psb.tile([P, 512], F32, tag="sc",
                                            name=f"s_{pr}_{kb}_{qi}")
                            # scores via one DoubleRow matmul: contraction
                            # over all 256 channels (two 128-planes).
                            nc.tensor.matmul(
                                s_ps[:],
                                qk[:, 2:4, kb * P:(kb + 1) * P],
                                qk[:, 0:2, qh],
                                start=True,
                                stop=True,
                                perf_mode=DR,
                            )
                            nc.scalar.activation(
                                pt[:, k2, qi * QT:(qi + 1) * QT],
                                s_ps[:], AF.Exp, scale=ATT_SCALE)
                        if kb == 0:
                            nc.vector.tensor_copy(s_acc[:], pt[:, 0])
                        else:
                            nc.vector.tensor_add(s_acc[:], s_acc[:],
                                                 pt[:, k2])
                    for ch in range(2):
                        vt = v_sb[:, 2 * kbp:2 * kbp + 2,
                                  ch * P:(ch + 1) * P]
                        for qi in range(2):
                            nc.tensor.matmul(
                                o_ps[qi][ch][:], vt,
                                pt[:, :, qi * QT:(qi + 1) * QT],
                                start=(kbp == 0), stop=(kbp == NKB // 2 - 1),
                                perf_mode=DR)
                    if kbp == 1 and pending is not None:
                        finalize_pair(pending)
                        pending = None
                pending = (pr, o_ps, s_acc)
            finalize_pair(pending)

    nc.finalize()
    return nc


# ---------------------------------------------------------------------------
# host side
# ---------------------------------------------------------------------------

def _prep_core_inputs(inputs, n_tok=H * W):
    """Build the per-core in_maps (shared weight tensors + per-core x)."""
    CCH = C // P
    f32 = np.float32
    bf16 = ml_dtypes.bfloat16

    x = np.asarray(inputs["x"], f32).reshape(B, C, n_tok)
    gn_scale = np.asarray(inputs["gn_scale"], f32)
    gn_bias = np.asarray(inputs["gn_bias"], f32)
    qkv_w = np.asarray(inputs["qkv_w"], f32)
    qkv_b = np.asarray(inputs["qkv_b"], f32)
    proj_w = np.asarray(inputs["proj_w"], f32)
    proj_b = np.asarray(inputs["proj_b"], f32)

    qkv_wt = np.ascontiguousarray(qkv_w.T).reshape(CCH, P, 3 * C).astype(bf16)
    qk_bias = qkv_b[:2 * C].reshape(4, P, 1).astype(f32).copy()
    v_bias = qkv_b[2 * C:].astype(f32)
    proj_wt = np.ascontiguousarray(proj_w.T).reshape(CCH, P, C).astype(bf16)
    proj_bt = proj_b.reshape(CCH, P, 1).astype(f32)
    gn_sc = gn_scale.reshape(CCH, P, 1).astype(f32)
    gn_bi = gn_bias.reshape(CCH, P, 1).astype(f32)

    ch = np.arange(C)
    gn_ind = np.zeros((CCH, P, P), f32)
    gn_ind[ch // P, ch % P, ch // (C // GROUPS)] = 1.0
    gn_ind2 = np.zeros((CCH, P, P), f32)
    for t in range(CCH):
        gn_ind2[t, :GROUPS, :] = gn_ind[t, :, :GROUPS].T

    shared = {
        "qkv_wt": qkv_wt,
        "qk_bias": qk_bias,
        "v_bias": v_bias,
        "proj_wt": proj_wt,
        "proj_b": proj_bt,
        "gn_sc": gn_sc,
        "gn_bi": gn_bi,
        "gn_ind": gn_ind,
        "gn_ind2": gn_ind2,
    }
    return [dict(shared, x=np.ascontiguousarray(x[i])) for i in range(B)]


_NC_CACHE = {}
LAST_RESULT = None  # BassKernelResults of the most recent run (for test.py)


def _get_nc():
    if "nc" not in _NC_CACHE:
        _NC_CACHE["nc"] = build_nc()
    return _NC_CACHE["nc"]


def kernel(**inputs) -> np.ndarray:
    global LAST_RESULT
    from concourse.bass_utils import run_bass_kernel_spmd

    nc = _get_nc()
    in_maps = _prep_core_inputs(inputs)
    res = run_bass_kernel_spmd(nc, in_maps, list(range(N_CORES)))
    LAST_RESULT = res
    out = np.stack([np.asarray(res.results[i]["out"]) for i in range(B)])
    return out.reshape(B, C, H, W).astype(np.float32)


# revision 27
# speedup vs baseline: 1.3892x; 1.0505x over previous
"""Trainium2 Bass kernel for nn_AttentionBlock (GroupNorm + single-head
self-attention + proj + residual), data-parallel over batch on 8 cores.

Contract: kernel(**inputs) takes the FULL unsharded inputs
  x (8, 256, 64, 64) f32, gn_scale (256,), gn_bias (256,),
  qkv_w (768, 256), qkv_b (768,), proj_w (256, 256), proj_b (256,)
and returns the FULL output (8, 256, 64, 64) f32.

Per-core plan (one sample per core):
  - x viewed as (C=256, N=4096) = (channels on partitions, tokens on free dim)
  - GroupNorm(8 groups) stats via bn_stats/bn_aggr + tiny indicator matmuls
  - xn cast to bf16; QKV as channel matmuls:
      Q, K produced in (d, n) layout  [d on partitions]
      V produced in token-major (n, d) layout
    so that attention needs NO transposes:
      scoresT[k, q] = sum_d K[d,k] Q[d,q]   (lhsT=K tile, rhs=Q tile)
      PT = exp(scoresT)                     (softmax scale folded into Q)
      out_un[d, q] = sum_k V[k,d] PT[k,q]   (lhsT=V tile, rhs=PT)
      denom[q] = sum_k PT[k,q]  -> DVE accumulation + one ones-matmul that
                                   also broadcasts the sum over partitions
  - out = proj(out_un * 1/denom) + proj_b + x  (residual in f32)
"""

import os
import sys

import numpy as np

for _p in (
    "/opt/trn_rl_repo",
    "/root/.axon_site",
    "/root/.axon_site/_ro/trn_rl_repo",
    "/root/.axon_site/_ro/pypackages",
):
    if os.path.isdir(_p) and _p not in sys.path:
        sys.path.append(_p)

import ml_dtypes  # noqa: E402

import concourse.bass as bass  # noqa: E402
import concourse.mybir as mybir  # noqa: E402
import concourse.tile as tile  # noqa: E402
from concourse import bacc  # noqa: E402

F32 = mybir.dt.float32
BF16 = mybir.dt.bfloat16
FP8 = mybir.dt.float8e4
AF = mybir.ActivationFunctionType
ALU = mybir.AluOpType
DR = mybir.MatmulPerfMode.DoubleRow

B, C, H, W = 8, 256, 64, 64
GROUPS = 8
EPS = 1e-5
P = 128
N_CORES = 8
ATT_SCALE = float(C) ** -0.5  # 1/16


def build_nc(n_tok=H * W):
    """Build the single-core Bass program (SPMD across 8 cores)."""
    CCH = C // P            # channel chunks (2)
    QT = 512                # q-tile width (one PSUM bank of f32)
    NQ = n_tok // QT        # number of q tiles
    NKB = n_tok // P        # number of 128-token key blocks
    GSZ = C // GROUPS       # channels per group (32)
    G_PER_CHUNK = GROUPS // CCH  # groups per 128-channel chunk (4)

    # Bacc (not plain Bass): its compile() runs move_matmul_waits_to_ldweights
    # + generate_event_semaphores, which split multi-wait matmuls to satisfy
    # the 1-sync-wait-per-instruction hardware constraint.
    nc = bacc.Bacc()

    # ---- DRAM I/O (per-core tensors; host shards batch over cores) ----
    x_d = nc.dram_tensor("x", [C, n_tok], F32, kind="ExternalInput")
    qkvw_d = nc.dram_tensor("qkv_wt", [CCH, P, 3 * C], BF16, kind="ExternalInput")
    qkbias_d = nc.dram_tensor("qk_bias", [4, P, 1], F32, kind="ExternalInput")
    vbias_d = nc.dram_tensor("v_bias", [C], F32, kind="ExternalInput")
    projw_d = nc.dram_tensor("proj_wt", [CCH, P, C], BF16, kind="ExternalInput")
    projb_d = nc.dram_tensor("proj_b", [CCH, P, 1], F32, kind="ExternalInput")
    gnsc_d = nc.dram_tensor("gn_sc", [CCH, P, 1], F32, kind="ExternalInput")
    gnbi_d = nc.dram_tensor("gn_bi", [CCH, P, 1], F32, kind="ExternalInput")
    # group-sum indicator (zero-padded to M=128 so the matmul avoids the
    # 32-column tile-mode lowering): ind[t, c, g] = (t*128 + c) // 32 == g
    gnind_d = nc.dram_tensor("gn_ind", [CCH, P, P], F32, kind="ExternalInput")
    # channel-broadcast indicator, padded to K=128: ind2[t, g, c] nonzero only g<8
    gnind2_d = nc.dram_tensor("gn_ind2", [CCH, P, P], F32, kind="ExternalInput")
    out_d = nc.dram_tensor("out", [C, n_tok], F32, kind="ExternalOutput")

    QP = 2 * QT             # paired q-tile width (1024)
    NQP = n_tok // QP       # number of q-tile pairs

    with tile.TileContext(nc) as tc:
        with (
            tc.tile_pool(name="persist", bufs=1) as pp,
            tc.tile_pool(name="work", bufs=3) as wp,
            tc.tile_pool(name="ps_sc", bufs=2, space="PSUM") as psb,
            tc.tile_pool(name="ps_o", bufs=1, space="PSUM") as pso,
        ):
            # ---------------- load weights / constants ----------------
            qkvw = pp.tile([P, CCH, 3 * C], BF16, tag="qkvw")
            nc.sync.dma_start(qkvw[:], qkvw_d.rearrange("t p o -> p t o"))
            projw = pp.tile([P, CCH, C], BF16, tag="projw")
            nc.sync.dma_start(projw[:], projw_d.rearrange("t p o -> p t o"))
            qkb = pp.tile([P, 4], F32, tag="qkb")
            nc.sync.dma_start(qkb[:], qkbias_d.rearrange("j p one -> p (j one)"))
            projb = pp.tile([P, CCH], F32, tag="projb")
            nc.sync.dma_start(projb[:], projb_d.rearrange("t p one -> p (t one)"))
            gnsc = pp.tile([P, CCH], F32, tag="gnsc")
            nc.sync.dma_start(gnsc[:], gnsc_d.rearrange("t p one -> p (t one)"))
            gnbi = pp.tile([P, CCH], F32, tag="gnbi")
            nc.sync.dma_start(gnbi[:], gnbi_d.rearrange("t p one -> p (t one)"))
            gnind = pp.tile([P, CCH, P], F32, tag="gnind")
            nc.sync.dma_start(gnind[:], gnind_d.rearrange("t p g -> p t g"))
            gnind2 = pp.tile([P, CCH, P], F32, tag="gnind2")
            nc.sync.dma_start(gnind2[:], gnind2_d.rearrange("t g c -> g t c"))
            # V bias broadcast across partitions (DMA with partition-stride 0)
            vbias = pp.tile([P, C], F32, tag="vbias")
            nc.sync.dma_start(vbias[:], vbias_d[None, :].to_broadcast([P, C]))
            # all-ones [128, 128] used to (sum over partitions + broadcast)
            ones_mat = pp.tile([P, P], BF16, tag="ones_mat")
            nc.vector.memset(ones_mat[:], 1.0)

            # ---------------- load x, GroupNorm stats ----------------
            x_sb = pp.tile([P, CCH, n_tok], F32, tag="x_sb")
            stats = pp.tile([P, CCH, 2], F32, tag="stats")
            for t in range(CCH):
                nc.sync.dma_start(x_sb[:, t], x_d[t * P:(t + 1) * P, :])
                bn6 = wp.tile([P, n_tok // 512, 6], F32, tag="bn6")
                xv = x_sb[:, t].rearrange("p (a b) -> p a b", b=512)
                for a in range(n_tok // 512):
                    nc.vector.bn_stats(bn6[:, a], xv[:, a])
                # mv = (mean, var) per partition
                nc.vector.bn_aggr(stats[:, t], bn6[:])
                # stats col1 := mean^2 + var = E[x^2] (col0 stays mean)
                nc.vector.scalar_tensor_tensor(
                    out=stats[:, t, 1:2],
                    in0=stats[:, t, 0:1],
                    scalar=stats[:, t, 0:1],
                    in1=stats[:, t, 1:2],
                    op0=ALU.mult,
                    op1=ALU.add,
                )

            # group aggregation: gagg[g, j] = sum_{c in group g} stats[c, j]
            gagg_ps = psb.tile([P, 2, 512], F32, tag="sc", name="gagg_ps")
            for t in range(CCH):
                nc.tensor.matmul(
                    gagg_ps[:, 0, :2],
                    gnind[:, t],
                    stats[:, t],
                    start=(t == 0),
                    stop=(t == CCH - 1),
                )
            # per-group a = rstd, b = -mean * rstd   (divide sums by GSZ first)
            gab = pp.tile([P, 2], F32, tag="gab")
            nc.vector.memset(gab[:], 0.0)
            gmean = wp.tile([P, 1], F32, tag="gmean")
            gtmp = wp.tile([P, 1], F32, tag="gtmp")
            nc.vector.tensor_scalar_mul(gmean[:GROUPS], gagg_ps[:GROUPS, 0, 0:1], 1.0 / GSZ)
            nc.vector.tensor_scalar_mul(gtmp[:GROUPS], gagg_ps[:GROUPS, 0, 1:2], 1.0 / GSZ)
            # gtmp := mean^2 - E[x^2] = -var
            nc.vector.scalar_tensor_tensor(
                out=gtmp[:GROUPS],
                in0=gmean[:GROUPS],
                scalar=gmean[:GROUPS],
                in1=gtmp[:GROUPS],
                op0=ALU.mult,
                op1=ALU.subtract,
            )
            # std = sqrt(-1 * gtmp + eps)
            epsb = wp.tile([P, 1], F32, tag="epsb")
            nc.vector.memset(epsb[:], EPS)
            nc.scalar.activation(gtmp[:GROUPS], gtmp[:GROUPS], AF.Sqrt,
                                 bias=epsb[:GROUPS], scale=-1.0)
            nc.vector.reciprocal(gab[:GROUPS, 0:1], gtmp[:GROUPS])  # a = rstd
            # b = -(mean * rstd):  (mean mult rstd) subtract 2*mean*rstd
            nc.vector.tensor_mul(gtmp[:GROUPS], gmean[:GROUPS], gab[:GROUPS, 0:1])
            nc.vector.tensor_scalar_mul(gab[:GROUPS, 1:2], gtmp[:GROUPS], -1.0)

            # broadcast (a, b) back to channels: chab[c, j] = gab[g(c), j]
            xn = pp.tile([P, CCH, n_tok], BF16, tag="xn")
            for t in range(CCH):
                chab_ps = psb.tile([P, 2, 512], F32, tag="sc", name=f"chab_ps{t}")[:, 0]
                nc.tensor.matmul(chab_ps[:, :2], gnind2[:, t], gab[:],
                                 start=True, stop=True)
                # mult_c = a * gamma_c ; add_c = b * gamma_c + beta_c
                chm = pp.tile([P, 1], F32, tag=f"chm{t}", name=f"chm{t}")
                cha = pp.tile([P, 1], F32, tag=f"cha{t}", name=f"cha{t}")
                nc.vector.tensor_mul(chm[:], chab_ps[:, 0:1], gnsc[:, t, None])
                nc.vector.scalar_tensor_tensor(
                    out=cha[:],
                    in0=chab_ps[:, 1:2],
                    scalar=gnsc[:, t, None],
                    in1=gnbi[:, t, None],
                    op0=ALU.mult,
                    op1=ALU.add,
                )
                # xn = x * mult + add   (bf16 out)
                nc.scalar.activation(xn[:, t], x_sb[:, t], AF.Identity,
                                     bias=cha[:], scale=chm[:])

            # ---------------- QKV ----------------
            # Q, K in (d, n) layout; j = 0,1 -> Q chunks; 2,3 -> K chunks.
            # softmax scale folded into Q (scale on PSUM->SBUF copy; host
            # pre-scaled the Q bias entries). N=1024 matmuls with bf16 PSUM
            # halve the matmul/copy instruction counts.
            # Q, K, V stored in fp8e4 (e4m3) for DoubleRow matmuls. The
            # softmax scale is NOT folded into Q here (it would push |q| to
            # ~0.02, into fp8 subnormals) — it moves into the exp() scale.
            qk = pp.tile([P, 4, n_tok], FP8, tag="qk")
            for j in range(4):
                for np2 in range(NQ // 2):
                    ns = slice(np2 * QP, (np2 + 1) * QP)
                    ps = psb.tile([P, 2, 512], F32, tag="sc",
                                  name=f"qk{j}_{np2}")
                    for half in range(2):
                        nsh = slice(np2 * QP + half * QT,
                                    np2 * QP + (half + 1) * QT)
                        for t in range(CCH):
                            nc.tensor.matmul(
                                ps[:, half],
                                qkvw[:, t, j * P:(j + 1) * P],
                                xn[:, t, nsh],
                                start=(t == 0),
                                stop=(t == CCH - 1),
                            )
                    nc.scalar.activation(
                        qk[:, j, ns], ps.rearrange("p a b -> p (a b)"),
                        AF.Identity,
                        bias=qkb[:, j, None],
                        scale=1.0,
                    )
            # V token-major: v_sb[:, kb, d] holds V[token kb*128+p, d]
            v_sb = pp.tile([P, NKB, C], FP8, tag="v_sb")
            for kbp in range(NKB // 2):
                ps = psb.tile([P, 2, 512], F32, tag="sc", name=f"v{kbp}")
                for k2 in range(2):
                    for t in range(CCH):
                        nc.tensor.matmul(
                            ps[:, k2, :C],
                            xn[:, t, (2 * kbp + k2) * P:(2 * kbp + k2 + 1) * P],
                            qkvw[:, t, 2 * C:3 * C],
                            start=(t == 0),
                            stop=(t == CCH - 1),
                        )
                nc.vector.tensor_add(
                    v_sb[:, 2 * kbp:2 * kbp + 2],
                    ps[:, :, :C],
                    vbias[:, None, :].to_broadcast([P, 2, C]),
                )

            # ---------------- attention + proj + residual ----------------
            # Processed in q-tile PAIRS (1024 q columns): scores/exp/s_acc
            # run pair-wide; PV stays per-qt (PSUM fp32 accumulation).
            # Each pair's finalize (denominator, normalize, proj, residual)
            # is DEFERRED into the next pair's kb loop so the PE never
            # stalls on the DVE chain at pair boundaries.
            def finalize_pair(st):
                pr, o_ps, s_accA, s_accB = st
                s_tot = wp.tile([P, QP], BF16, tag="s_tot", bufs=2)
                nc.vector.tensor_add(s_tot[:], s_accA[:], s_accB[:])
                rec = wp.tile([P, QP], F32, tag="rec", bufs=2)
                obs = []
                for qi in range(2):
                    qh = slice(qi * QT, (qi + 1) * QT)
                    den = psb.tile([P, 2, 512], F32, tag="sc",
                                   name=f"den_{pr}_{qi}")[:, 0]
                    nc.tensor.matmul(den[:], ones_mat[:], s_tot[:, qh],
                                     start=True, stop=True)
                    nc.vector.reciprocal_approx_fast(rec[:, qh], den[:])
                for ch in range(2):
                    ob = wp.tile([P, QP], BF16, tag=f"ob{ch}", bufs=2,
                                 name=f"ob{ch}_{pr}")
                    for qi in range(2):
                        qh = slice(qi * QT, (qi + 1) * QT)
                        nc.vector.tensor_mul(ob[:, qh], o_ps[qi][ch][:],
                                             rec[:, qh])
                    obs.append(ob)
                for t in range(CCH):
                    for qi in range(2):
                        qh = slice(qi * QT, (qi + 1) * QT)
                        qg = slice(pr * QP + qi * QT, pr * QP + (qi + 1) * QT)
                        p_ps = psb.tile([P, 512], F32, tag="sc",
                                        name=f"p_{pr}_{t}_{qi}")
                        nc.tensor.matmul(p_ps[:],
                                         projw[:, 0, t * P:(t + 1) * P],
                                         obs[0][:, qh], start=True, stop=False)
                        nc.tensor.matmul(p_ps[:],
                                         projw[:, 1, t * P:(t + 1) * P],
                                         obs[1][:, qh], start=False, stop=True)
                        res = wp.tile([P, QT], F32, tag="res", bufs=3)
                        nc.vector.scalar_tensor_tensor(
                            out=res[:],
                            in0=p_ps[:],
                            scalar=projb[:, t, None],
                            in1=x_sb[:, t, qg],
                            op0=ALU.add,
                            op1=ALU.add,
                        )
                        nc.sync.dma_start(out_d[t * P:(t + 1) * P, qg], res[:])

            pending = None
            for pr in range(NQP):
                qps = slice(pr * QP, (pr + 1) * QP)
                o_ps = [[pso.tile([P, 512], F32, tag=f"o{qi}{ch}",
                                  name=f"o{qi}{ch}_{pr}")
                         for ch in range(2)] for qi in range(2)]
                # two running denominator accumulators: even key blocks on
                # the vector engine, odd ones on the (otherwise idle) gpsimd
                # engine; combined (in bf16) at finalize time.
                s_accA = wp.tile([P, QP], F32, tag="s_accA", bufs=2)
                s_accB = wp.tile([P, QP], F32, tag="s_accB", bufs=2)
                for kbp in range(NKB // 2):
                    # pt holds exp(scores) for the TWO key blocks of this
                    # DoubleRow pair: plane i = key block kbp*2+i (fp8).
                    pt = wp.tile([P, 2, QP], FP8, tag="pt", bufs=3)
                    for k2 in range(2):
                        kb = 2 * kbp + k2
                        s_ps = psb.tile([P, 2, 512], F32, tag="sc",
                                        name=f"s_{pr}_{kb}")
                        for qi in range(2):
                            qh = slice(pr * QP + qi * QT,
                                       pr * QP + (qi + 1) * QT)
                            # scores via one DoubleRow matmul: contraction
                            # over all 256 channels (two 128-planes).
                            nc.tensor.matmul(
                                s_ps[:, qi],
                                qk[:, 2:4, kb * P:(kb + 1) * P],
                                qk[:, 0:2, qh],
                                start=True,
                                stop=True,
                                perf_mode=DR,
                            )
                        nc.scalar.activation(
                            pt[:, k2],
                            s_ps.rearrange("p a b -> p (a b)"),
                            AF.Exp, scale=ATT_SCALE)
                        if kb == 0:
                            nc.vector.tensor_copy(s_accA[:], pt[:, 0])
                        elif kb == 1:
                            nc.gpsimd.tensor_copy(s_accB[:], pt[:, 1])
                        elif k2 == 0:
                            nc.vector.tensor_add(s_accA[:], s_accA[:],
                                                 pt[:, 0])
                        else:
                            nc.gpsimd.tensor_add(s_accB[:], s_accB[:],
                                                 pt[:, 1])
                    for ch in range(2):
                        vt = v_sb[:, 2 * kbp:2 * kbp + 2,
                                  ch * P:(ch + 1) * P]
                        for qi in range(2):
                            nc.tensor.matmul(
                                o_ps[qi][ch][:], vt,
                                pt[:, :, qi * QT:(qi + 1) * QT],
                                start=(kbp == 0), stop=(kbp == NKB // 2 - 1),
                                perf_mode=DR)
                    if kbp == 1 and pending is not None:
                        finalize_pair(pending)
                        pending = None
                pending = (pr, o_ps, s_accA, s_accB)
            finalize_pair(pending)

    nc.finalize()
    return nc


# ---------------------------------------------------------------------------
# host side
# ---------------------------------------------------------------------------

def _prep_core_inputs(inputs, n_tok=H * W):
    """Build the per-core in_maps (shared weight tensors + per-core x)."""
    CCH = C // P
    f32 = np.float32
    bf16 = ml_dtypes.bfloat16

    x = np.asarray(inputs["x"], f32).reshape(B, C, n_tok)
    gn_scale = np.asarray(inputs["gn_scale"], f32)
    gn_bias = np.asarray(inputs["gn_bias"], f32)
    qkv_w = np.asarray(inputs["qkv_w"], f32)
    qkv_b = np.asarray(inputs["qkv_b"], f32)
    proj_w = np.asarray(inputs["proj_w"], f32)
    proj_b = np.asarray(inputs["proj_b"], f32)

    qkv_wt = np.ascontiguousarray(qkv_w.T).reshape(CCH, P, 3 * C).astype(bf16)
    qk_bias = qkv_b[:2 * C].reshape(4, P, 1).astype(f32).copy()
    v_bias = qkv_b[2 * C:].astype(f32)
    proj_wt = np.ascontiguousarray(proj_w.T).reshape(CCH, P, C).astype(bf16)
    proj_bt = proj_b.reshape(CCH, P, 1).astype(f32)
    gn_sc = gn_scale.reshape(CCH, P, 1).astype(f32)
    gn_bi = gn_bias.reshape(CCH, P, 1).astype(f32)

    ch = np.arange(C)
    gn_ind = np.zeros((CCH, P, P), f32)
    gn_ind[ch // P, ch % P, ch // (C // GROUPS)] = 1.0
    gn_ind2 = np.zeros((CCH, P, P), f32)
    for t in range(CCH):
        gn_ind2[t, :GROUPS, :] = gn_ind[t, :, :GROUPS].T

    shared = {
        "qkv_wt": qkv_wt,
        "qk_bias": qk_bias,
        "v_bias": v_bias,
        "proj_wt": proj_wt,
        "proj_b": proj_bt,
        "gn_sc": gn_sc,
        "gn_bi": gn_bi,
        "gn_ind": gn_ind,
        "gn_ind2": gn_ind2,
    }
    return [dict(shared, x=np.ascontiguousarray(x[i])) for i in range(B)]


_NC_CACHE = {}
LAST_RESULT = None  # BassKernelResults of the most recent run (for test.py)


def _get_nc():
    if "nc" not in _NC_CACHE:
        _NC_CACHE["nc"] = build_nc()
    return _NC_CACHE["nc"]


def kernel(**inputs) -> np.ndarray:
    global LAST_RESULT
    from concourse.bass_utils import run_bass_kernel_spmd

    nc = _get_nc()
    in_maps = _prep_core_inputs(inputs)
    res = run_bass_kernel_spmd(nc, in_maps, list(range(N_CORES)))
    LAST_RESULT = res
    out = np.stack([np.asarray(res.results[i]["out"]) for i in range(B)])
    return out.reshape(B, C, H, W).astype(np.float32)
